# revision 16
# baseline (speedup 1.0000x reference)
"""Trainium2 Bass kernel for the EnsembleGRU problem (8-core SPMD).

Math (per ensemble e, flattened batch n, timestep w):
    y  = x @ weight_linear.T + bias_linear          (P=72 proj)
    gx = y @ w_ih.T + b_ih                          (3 gates)
composes to gx = x @ W_eff.T + b_eff with
    W_eff[e,g,f] = sum_p w_ih[e,g,p] * weight_linear[e,p,f]
    b_eff[e,g]   = sum_p w_ih[e,g,p] * bias_linear[e,p] + b_ih[e,g]
then the GRU (hidden_size=1) scan:
    r = sigmoid(gx0 + w0*h + c0);  z = sigmoid(gx1 + w1*h + c1)
    n = tanh(gx2 + bn + r*(w2*h + b2));  h' = h + (1-z)*(n - h)

Device plan per core (2 ensembles, 1024 chains = 128 partitions x 8 cols):
  - x is pre-transposed and f16-cast on host to [W, (e,f)=128, n=512],
    so the DMA feeds the PE directly (no on-device cast/transpose).
  - PE: per step, 16 matmuls [128K x 64M] @ [128K x 3N] -> PSUM gx
    [128, 24] (cols = 8g + j), chain layout p = 64e + n%64, j = n//64.
  - recurrence per step, restructured around the carrier pair
    (q1, s1n) with h = q1 - s1n, q1 = zc*n, s1n = (zc-1)*h_prev
    (gate-z sign-flipped on host so sigmoid gives zc = 1-z).
    Every use of h is affine in q1 with an m2-term precomputed from s1n
    off the critical path, so the path per step is only
    q1 -> a_r -> sigmoid -> v -> t -> tanh -> q1':
      DVE  a_r = q1*w0 + m2r;     ACT r  = sigmoid(a_r + c0)
      DVE  h_out = q1 - s1n       -> hall slice              (off path)
      DVE  a_z = h_out*w1n + gx1n; ACT zc = sigmoid(a_z + c1n)  (off path)
      DVE  v   = (h_out*w2 + b2) * r   (affine_mul_reduce)
      DVE  t   = v + gx2;  ACT n = tanh(t + bn)
      DVE  s1n' = (zc - 1) * h_out                           (off path)
      DVE  m2r' = gx0' - w0*s1n'                             (off path)
      DVE  q1' = zc * n
  - h history accumulates in SBUF (hall [128, 8*W]); one output DMA at
    the end (keeps per-step DMA issues off the ACT engine).
"""
import numpy as np
from contextlib import ExitStack

W_STEPS, E, B, I, F = 128, 16, 64, 8, 64
N = B * I            # 512
E_LOC = 2            # ensembles per core
N_CORES = 8
PSUM_SLOTS = 6       # gx pipeline depth (one slot per PSUM bank)
NJ = 8               # chain columns (n // 64)


def _chain_maps():
    """e_idx, n_idx arrays [128, 8] for (p, j) -> (e_loc, n)."""
    p = np.arange(128)[:, None]
    j = np.arange(NJ)[None, :]
    e_idx = np.broadcast_to(p // 64, (128, NJ)).astype(np.int64)
    n_idx = (64 * j + p % 64).astype(np.int64)
    return e_idx, np.broadcast_to(n_idx, (128, NJ))


_E_IDX, _N_IDX = _chain_maps()


def _build_program(n_steps=W_STEPS, loop=1, mode="full"):
    import concourse.bass as bass
    import concourse.tile as tile
    from concourse import bacc, mybir
    from concourse.alu_op_type import AluOpType

    nc = bacc.Bacc("TRN2", num_devices=N_CORES)
    f32, f16 = mybir.dt.float32, mybir.dt.float16
    AF = mybir.ActivationFunctionType

    # ---- DRAM I/O ----
    xt = nc.dram_tensor("xt", [n_steps, 128, N], f16, kind="ExternalInput").ap()
    we16 = nc.dram_tensor("we16", [128, 6], f16, kind="ExternalInput").ap()
    scb = nc.dram_tensor("scb", [128, 12], f32, kind="ExternalInput").ap()
    h0in = nc.dram_tensor("h0in", [128, NJ], f32, kind="ExternalInput").ap()
    hout = nc.dram_tensor("hout", [128, n_steps * NJ], f32, kind="ExternalOutput").ap()

    with tile.TileContext(nc) as tc, ExitStack() as ctx:
        cpool = ctx.enter_context(tc.tile_pool(name="consts", bufs=1))
        xp = ctx.enter_context(tc.tile_pool(name="x", bufs=PSUM_SLOTS))
        gp = ctx.enter_context(tc.tile_pool(name="g", bufs=4))
        rp = ctx.enter_context(tc.tile_pool(name="r", bufs=3))

        # constants
        we = cpool.tile([128, 6], f16, name="we")
        nc.sync.dma_start(we[:], we16[:])
        sc = cpool.tile([128, 12], f32, name="sc")
        nc.sync.dma_start(sc[:], scb[:])
        w0v, w1nv, w2v = sc[:, 0:1], sc[:, 1:2], sc[:, 2:3]
        c0v, c1nv, b2v, bnv = sc[:, 3:4], sc[:, 4:5], sc[:, 5:6], sc[:, 6:7]
        w0nv, w1nnv, w2nv = sc[:, 7:8], sc[:, 8:9], sc[:, 9:10]

        h0 = cpool.tile([128, NJ], f32, name="h0")
        nc.sync.dma_start(h0[:], h0in[:])

        # h history / output accumulator: [128, n_steps * 8]
        hall = cpool.tile([128, n_steps * NJ], f32, name="hall")

        ps_banks = [nc.place_psum_tensor(f"gx{b}", [128, 24], f32, bank=b)
                    for b in range(PSUM_SLOTS)]

        n_tot = loop * n_steps
        gxs_tiles = {}

        def stage(wg):
            """DMA x[w], run matmuls into PSUM, Pool-copy gx to SBUF."""
            w = wg % n_steps
            ps = ps_banks[wg % PSUM_SLOTS].ap()
            ps3 = ps.rearrange("p (g j) -> p j g", g=3, j=NJ)  # col = 8g + j
            xw = xp.tile([128, N], f16, name="xw")
            nc.sync.dma_start(xw[:], xt[w])
            for e in range(E_LOC):
                for j in range(NJ):
                    nc.tensor.matmul(ps3[64 * e:64 * e + 64, j, :],
                                     xw[:, 64 * j:64 * j + 64],
                                     we[:, 3 * e:3 * e + 3])
            gxs = gp.tile([128, 3 * NJ], f32, name="gxs")
            nc.scalar.copy(gxs[:], ps[:])  # Pool/GPSIMD cannot access PSUM
            gxs_tiles[wg] = gxs

        # prologue: s1n(-1) = 0, q1(-1) = h0 (so h(-1) = q1 - s1n = h0)
        s1n_prev = rp.tile([128, NJ], f32, name="s1n0")
        nc.vector.memzero(s1n_prev[:])
        q1_prev = h0[:]
        h_prev_out = h0[:]          # h(w-1) materialized
        stage(0)
        gxs0 = gxs_tiles.pop(0)
        # m2r for step 0 (s1n_prev = 0, so it is just gx0 + 0)
        m2r = rp.tile([128, NJ], f32, name="m2r")
        nc.vector.scalar_tensor_tensor(m2r[:], s1n_prev[:], w0nv, gxs0[:, 0:NJ],
                                       AluOpType.mult, AluOpType.add)
        gxs_cur = gxs0

        for wg in range(n_tot):
            if wg + 1 < n_tot:
                stage(wg + 1)

            # --- critical path head: a_r from q1 directly ---
            a_r = rp.tile([128, NJ], f32, name="a_r")
            nc.vector.scalar_tensor_tensor(a_r[:], q1_prev, w0v, m2r[:],
                                           AluOpType.mult, AluOpType.add)
            # materialize h(w-1) into hall (feeds a_z, v, s1n)
            if wg > 0:
                h_out = hall[:, NJ * ((wg - 1) % n_steps):NJ * ((wg - 1) % n_steps) + NJ]
                nc.vector.tensor_tensor(h_out, q1_prev, s1n_prev[:], AluOpType.subtract)
                h_prev_out = h_out
            a_z = rp.tile([128, NJ], f32, name="a_z")
            nc.vector.scalar_tensor_tensor(a_z[:], h_prev_out, w1nv, gxs_cur[:, NJ:2 * NJ],
                                           AluOpType.mult, AluOpType.add)
            r_t = rp.tile([128, NJ], f32, name="r_t")
            nc.scalar.activation(r_t[:], a_r[:], AF.Sigmoid, bias=c0v)
            zc = rp.tile([128, NJ], f32, name="zc")
            nc.scalar.activation(zc[:], a_z[:], AF.Sigmoid, bias=c1nv)

            v = rp.tile([128, NJ], f32, name="v")
            acc = rp.tile([128, 1], f32, name="acc")
            nc.vector.affine_mul_reduce(v[:], acc[:], h_prev_out, r_t[:], w2v, b2v)
            t = rp.tile([128, NJ], f32, name="t")
            nc.vector.tensor_tensor(t[:], v[:], gxs_cur[:, 2 * NJ:3 * NJ], AluOpType.add)
            n_t = rp.tile([128, NJ], f32, name="n_t")
            nc.scalar.activation(n_t[:], t[:], AF.Tanh, bias=bnv)

            s1n = rp.tile([128, NJ], f32, name="s1n")
            nc.vector.scalar_tensor_tensor(s1n[:], zc[:], 1.0, h_prev_out,
                                           AluOpType.subtract, AluOpType.mult)
            if wg + 1 < n_tot:
                gxs_nxt = gxs_tiles.pop(wg + 1)
                m2r = rp.tile([128, NJ], f32, name="m2r")
                nc.vector.scalar_tensor_tensor(m2r[:], s1n[:], w0nv, gxs_nxt[:, 0:NJ],
                                               AluOpType.mult, AluOpType.add)
            q1 = rp.tile([128, NJ], f32, name="q1")
            nc.vector.tensor_tensor(q1[:], zc[:], n_t[:], AluOpType.mult)
            q1_prev = q1[:]
            s1n_prev = s1n
            if wg + 1 < n_tot:
                gxs_cur = gxs_nxt

        # epilogue: final h
        h_out = hall[:, NJ * ((n_tot - 1) % n_steps):NJ * ((n_tot - 1) % n_steps) + NJ]
        nc.vector.tensor_tensor(h_out, q1_prev, s1n_prev[:], AluOpType.subtract)

        nc.sync.dma_start(hout[:], hall[:])

    nc.compile()
    return nc


_PROGRAM_CACHE = {}


def _get_program(n_steps=W_STEPS, loop=1, mode="full"):
    key = (n_steps, loop, mode)
    if key not in _PROGRAM_CACHE:
        _PROGRAM_CACHE[key] = _build_program(n_steps, loop, mode)
    return _PROGRAM_CACHE[key]


def _host_prep(inputs, state, weight_linear, bias_linear, w_ih, w_hh, b_ih, b_hh):
    """Per-core input maps."""
    n_steps = inputs.shape[0]
    W_eff = np.einsum("egp,epf->egf", w_ih.astype(np.float64), weight_linear.astype(np.float64))
    b_eff = np.einsum("egp,ep->eg", w_ih.astype(np.float64), bias_linear.astype(np.float64)) + b_ih
    W_eff = W_eff.astype(np.float32)
    b_eff = b_eff.astype(np.float32)

    x = inputs.reshape(n_steps, E, N, F)
    h_state = state[-1].reshape(E, N).astype(np.float32)

    in_maps = []
    for k in range(N_CORES):
        es = [2 * k, 2 * k + 1]
        # x slice -> [W, (e,f), n] f16 (pre-transposed for direct PE feed)
        xs = x[:, es].transpose(0, 1, 3, 2).reshape(n_steps, 128, N)
        xs = np.ascontiguousarray(xs, dtype=np.float16)

        # weight stacks [128 (e,f), 6] f16; z-gate negated so sigmoid gives 1-z
        we = np.zeros((128, 6), np.float16)
        wsign = np.array([1.0, -1.0, 1.0], np.float32)
        we[0:64, 0:3] = (W_eff[es[0]] * wsign[:, None]).T.astype(np.float16)   # [f, g]
        we[64:128, 3:6] = (W_eff[es[1]] * wsign[:, None]).T.astype(np.float16)

        # per-partition scale/bias vectors [128, 12]
        erow = np.repeat(np.array(es), 64)  # 128 rows -> global e
        scb = np.zeros((128, 12), np.float32)
        scb[:, 0] = w_hh[erow, 0]
        scb[:, 1] = -w_hh[erow, 1]
        scb[:, 2] = w_hh[erow, 2]
        scb[:, 3] = b_eff[erow, 0] + b_hh[erow, 0]
        scb[:, 4] = -(b_eff[erow, 1] + b_hh[erow, 1])
        scb[:, 5] = b_hh[erow, 2]
        scb[:, 6] = b_eff[erow, 2]
        scb[:, 7] = -scb[:, 0]   # -w0
        scb[:, 8] = -scb[:, 1]   # -w1n
        scb[:, 9] = -scb[:, 2]   # -w2

        # h0 in chain layout [128, 8]
        h0 = h_state[2 * k + _E_IDX, _N_IDX].astype(np.float32)

        in_maps.append({"xt": xs, "we16": we, "scb": scb, "h0in": h0})
    return in_maps


def _unpack_outputs(results):
    """results: list of dicts with 'hout' [128, W*8] -> full (W, E, B, I, 1)."""
    out = np.zeros((W_STEPS, E, N), np.float32)
    for k in range(N_CORES):
        h = results[k]["hout"].reshape(128, W_STEPS, NJ)  # [p, w, j]
        out[:, 2 * k + _E_IDX, _N_IDX] = h.transpose(1, 0, 2)
    return out.reshape(W_STEPS, E, B, I, 1)


def kernel(inputs, state, weight_linear, bias_linear, w_ih, w_hh, b_ih, b_hh):
    from concourse.bass_utils import run_bass_kernel_spmd

    nc = _get_program()
    in_maps = _host_prep(np.asarray(inputs, np.float32), np.asarray(state, np.float32),
                         np.asarray(weight_linear, np.float32), np.asarray(bias_linear, np.float32),
                         np.asarray(w_ih, np.float32), np.asarray(w_hh, np.float32),
                         np.asarray(b_ih, np.float32), np.asarray(b_hh, np.float32))
    res = run_bass_kernel_spmd(nc, in_maps, core_ids=list(range(N_CORES)))
    return _unpack_outputs(res.results)
